# revision 1
# baseline (speedup 1.0000x reference)
"""Trainium2 Bass kernel for nn_ClassAtt (dense MLP + 3-way class attention).

Model (per row of tube [B, 1536]):
  x1,x2,x3 = tube split into 3x512
  P_i   = relu(x_i @ w_i.T + b_i)            [B, 1024]
  last  = relu(concat(P1,P2,P3) @ wh.T + bh) [B, 1024]
  a_i   = rowwise_dot(last, P_i); w = softmax(a)  [B, 3]
  ctx   = sum_i w_i * P_i                    [B, 1024]
  out   = relu(concat(ctx, last) @ wd1.T + bd1) @ wd2.T + bd2  [B, 1000]

Strategy: pure data parallel over 8 NeuronCores (2048 rows each).  All
activations live in transposed [feature, row] layout so the contraction dim
is always on SBUF partitions and biases are per-partition scalars.  Weights
are host-transposed to [K, F].  Matmuls run as float32r (full PE rate for
free dim >= 256, ~1e-4 relative rounding) with fp32 PSUM accumulation.
Phases (DRAM spills between them):
  P1: L1 (3x 512->1024) -> PT spill   [whT chunk-loads interleaved into P1]
  P2: L2 (3072->1024) + attention -> dec spill (= [ctx; last])
  F0/F1: decode split by contraction half: out_h = relu(dec @ wd1_h.T
        + bd1_h) @ wd2_h.T; host sums the two halves (+bd2 in half 1).
DMA ordering matters: weight tensors are loaded with per-chunk DMAs
interleaved after the activation loads they must not delay (HWDGE is FIFO
per issuing engine).  The attention elementwise products run on GpSimd to
keep VectorE off the critical path.
"""

import numpy as np

import concourse.bass as bass
import concourse.mybir as mybir
import concourse.tile as tile
from concourse import bacc
from concourse.bass_utils import run_bass_kernel_spmd

F32 = mybir.dt.float32
F32R = mybir.dt.float32r

N_CORES = 8
B = 16384
ROWS = B // N_CORES  # rows per core
M = 1024             # hidden width
DEC_H = 2048
OUT = 1000

AluOp = mybir.AluOpType
Act = mybir.ActivationFunctionType


def build_nc(mm_dtype=F32R):
    nc = bacc.Bacc(None, target_bir_lowering=False)

    # ---- DRAM I/O (per-core shapes) ----
    xT = nc.dram_tensor("xT", [12, 128, ROWS], mm_dtype, kind="ExternalInput")
    wT = [
        nc.dram_tensor(f"w{i + 1}T", [4, 128, M], mm_dtype, kind="ExternalInput")
        for i in range(3)
    ]
    whT = nc.dram_tensor("whT", [24, 128, M], mm_dtype, kind="ExternalInput")
    wd1T = nc.dram_tensor("wd1T", [16, 128, DEC_H], mm_dtype, kind="ExternalInput")
    wd2T = nc.dram_tensor("wd2T", [16, 128, OUT], mm_dtype, kind="ExternalInput")
    bv = [
        nc.dram_tensor(f"b{i + 1}", [128, 8], F32, kind="ExternalInput")
        for i in range(3)
    ]
    bh = nc.dram_tensor("bh", [128, 8], F32, kind="ExternalInput")
    bd1 = nc.dram_tensor("bd1", [128, 16], F32, kind="ExternalInput")
    bd2 = nc.dram_tensor("bd2", [128, 8], F32, kind="ExternalInput")
    outH = [
        nc.dram_tensor(f"out{h}", [OUT, ROWS], F32, kind="ExternalOutput")
        for h in range(2)
    ]

    with tile.TileContext(nc) as tc:
        with tc.tile_pool(name="dram", bufs=1, space="DRAM") as dram:
            PT = dram.tile([3, 8, 128, ROWS], mm_dtype)  # P_i transposed
            dec = dram.tile([8, 128, ROWS], mm_dtype)    # last, transposed
            WS = dram.tile([128, 3, ROWS], mm_dtype)     # softmax weights

            # p2w outlives phase 1 so whT streams in during P1's compute.
            with (
                tc.tile_pool(name="p2w", bufs=1) as p2w,
                tc.tile_pool(name="psA", bufs=3, space="PSUM") as psA,
            ):
                wh_sb = p2w.tile([128, 24, M], mm_dtype)
                bh_sb = p2w.tile([128, 8], F32, tag="bh")
                ones_f32 = p2w.tile([128, 128], F32, tag="ones_f32")
                ones_sb = p2w.tile([128, 128], mm_dtype, tag="ones")

                # ------------ Phase 1: P_i = relu(x_i @ w_i.T + b_i) --------
                R1 = 256
                NT1 = ROWS // R1
                with (
                    tc.tile_pool(name="p1w", bufs=1) as p1w,
                    tc.tile_pool(name="p1x", bufs=2) as p1x,
                    tc.tile_pool(name="p1e", bufs=3) as p1e,
                ):
                    # xt tiles created lazily, loads interleaved with weights
                    xts = {}

                    def load_xt(rt):
                        rs = slice(rt * R1, (rt + 1) * R1)
                        t = p1x.tile([128, 12, R1], mm_dtype, tag="xt",
                                     name="xt")
                        for i in range(3):
                            nc.sync.dma_start(
                                t[:, 4 * i:4 * i + 4, :],
                                xT.ap()[4 * i:4 * i + 4, :, rs]
                                .rearrange("c p r -> p c r"),
                            )
                        xts[rt] = t

                    w_sb = []
                    b_sb = []
                    for i in range(3):
                        w = p1w.tile([128, 4, M], mm_dtype, tag=f"w{i}",
                                     name=f"w{i}")
                        nc.scalar.dma_start(w, wT[i].ap().rearrange("c p f -> p c f"))
                        b = p1w.tile([128, 8], F32, tag=f"b{i}", name=f"b{i}")
                        nc.scalar.dma_start(b, bv[i].ap())
                        w_sb.append(w)
                        b_sb.append(b)
                        if i < 2:
                            load_xt(i)  # first row-tiles right behind w1
                    nc.scalar.dma_start(bh_sb, bh.ap())
                    nc.any.memset(ones_f32, 1.0)
                    nc.vector.tensor_copy(ones_sb, ones_f32)

                    for rt in range(NT1):
                        # stream 3 whT chunks per row-tile behind xt prefetch
                        if rt + 2 < NT1:
                            load_xt(rt + 2)
                        for c in range(3 * rt, 3 * rt + 3):
                            nc.scalar.dma_start(wh_sb[:, c, :], whT.ap()[c])
                        rs = slice(rt * R1, (rt + 1) * R1)
                        xt = xts.pop(rt)
                        for i in range(3):
                            ev = p1e.tile([128, 8, R1], mm_dtype)
                            for fc in range(8):
                                ps = psA.tile([128, R1], F32, tag="mm",
                                              name="ps1")
                                for kc in range(4):
                                    nc.tensor.matmul(
                                        ps,
                                        w_sb[i][:, kc, fc * 128:(fc + 1) * 128],
                                        xt[:, i * 4 + kc, :],
                                        start=(kc == 0),
                                        stop=(kc == 3),
                                    )
                                nc.vector.tensor_scalar(
                                    ev[:, fc, :], ps, b_sb[i][:, fc:fc + 1],
                                    0.0, AluOp.add, AluOp.max,
                                )
                            nc.sync.dma_start(
                                PT[i, :, :, rs].rearrange("c p r -> p c r"), ev
                            )

                # ------ Phase 2: last = relu(hid1 @ wh.T + bh); attention ---
                R2 = 256
                with (
                    tc.tile_pool(name="p2pt", bufs=2) as p2pt,
                    tc.tile_pool(name="p2last", bufs=2) as p2last,
                    tc.tile_pool(name="p2big", bufs=1) as p2big,
                    tc.tile_pool(name="p2sm", bufs=1) as p2sm,
                    tc.tile_pool(name="psC", bufs=5, space="PSUM") as psC,
                ):
                    for rt in range(ROWS // R2):
                        rs = slice(rt * R2, (rt + 1) * R2)
                        pt = []
                        for i in range(3):
                            pt_i = p2pt.tile([128, 8, R2], mm_dtype,
                                             tag=f"pt{i}", name=f"pt{i}")
                            nc.sync.dma_start(
                                pt_i, PT[i, :, :, rs].rearrange("c p r -> p c r")
                            )
                            pt.append(pt_i)
                        last = p2last.tile([128, 8, R2], mm_dtype)
                        for fc in range(8):
                            ps = psA.tile([128, R2], F32, tag="mm", name="ps2")
                            for i in range(3):
                                for kc in range(8):
                                    nc.tensor.matmul(
                                        ps,
                                        wh_sb[:, i * 8 + kc,
                                              fc * 128:(fc + 1) * 128],
                                        pt[i][:, kc, :],
                                        start=(i == 0 and kc == 0),
                                        stop=(i == 2 and kc == 7),
                                    )
                            nc.scalar.activation(
                                last[:, fc, :], ps, Act.Relu,
                                bias=bh_sb[:, fc:fc + 1],
                            )
                        nc.sync.dma_start(
                            dec[:, :, rs].rearrange("c p r -> p c r"), last
                        )

                        # alphas: partition-sum of last*P_i via ones-matmul
                        # (partition-redundant [128, R2])
                        aps = []
                        for i in range(3):
                            tmp = p2big.tile([128, 8, R2], mm_dtype,
                                             tag="tmp", name=f"tmp{i}",
                                             bufs=2)
                            eng = nc.gpsimd if i == 2 else nc.vector
                            eng.tensor_tensor(tmp, last, pt[i], AluOp.mult)
                            ap_i = psC.tile([128, R2], F32, tag="alpha",
                                            name=f"alpha{i}")
                            for fc in range(8):
                                nc.tensor.matmul(
                                    ap_i, ones_sb, tmp[:, fc, :],
                                    start=(fc == 0), stop=(fc == 7),
                                )
                            aps.append(ap_i)

                        # batched softmax over the 3 logits -> WS spill
                        asb = p2sm.tile([128, 3, R2], F32, tag="asb")
                        for i in range(3):
                            nc.scalar.copy(asb[:, i, :], aps[i])
                        ai = asb.rearrange("p i r -> p r i")
                        mx = p2sm.tile([128, R2], F32, tag="mx")
                        nc.vector.reduce_max(mx, ai, axis=mybir.AxisListType.X)
                        bshp = (128, 3, R2)
                        nc.vector.tensor_tensor(
                            asb, asb, mx[:, None, :].to_broadcast(bshp),
                            AluOp.subtract)
                        nc.scalar.activation(asb, asb, Act.Exp)
                        ssum = p2sm.tile([128, R2], F32, tag="ssum")
                        nc.vector.reduce_sum(ssum, ai, axis=mybir.AxisListType.X)
                        rcp = p2sm.tile([128, R2], F32, tag="rcp")
                        nc.vector.reciprocal(rcp, ssum)
                        wsr = p2sm.tile([128, 3, R2], mm_dtype, tag="wsr")
                        nc.vector.tensor_tensor(
                            wsr, asb, rcp[:, None, :].to_broadcast(bshp),
                            AluOp.mult)
                        nc.sync.dma_start(WS[:, :, rs], wsr)

            # ---- Decode: out_h = relu(dec @ wd1_h.T + bd1_h) @ wd2_h.T -----
            RF = 256
            NTF = ROWS // RF
            for h in range(2):
                with (
                    tc.tile_pool(name=f"fw{h}", bufs=1) as fw,
                    tc.tile_pool(name=f"fd{h}", bufs=3) as fd,
                    tc.tile_pool(name=f"fo{h}", bufs=2) as fo,
                    tc.tile_pool(name=f"fe{h}", bufs=2) as fe,
                    tc.tile_pool(name=f"psF{h}", bufs=4, space="PSUM") as psF,
                    tc.tile_pool(name=f"psG{h}", bufs=4, space="PSUM") as psG,
                ):
                    dcs = {}

                    def load_dc(rt, fd=fd):
                        rs = slice(rt * RF, (rt + 1) * RF)
                        t = fd.tile([128, 16, RF], mm_dtype, tag="dc",
                                    name="dc", bufs=2)
                        nc.sync.dma_start(
                            t[:, 8:16, :], dec[:, :, rs].rearrange("c p r -> p c r")
                        )
                        wf = fd.tile([128, 3, RF], mm_dtype, tag="wf",
                                     name="wf", bufs=2)
                        nc.sync.dma_start(wf, WS[:, :, rs])
                        dcs[rt] = (t, wf)

                    wd1_sb = fw.tile([128, 16, M], mm_dtype, tag="wd1")
                    wd2_sb = fw.tile([128, 8, OUT], mm_dtype, tag="wd2")
                    bd1_sb = fw.tile([128, 8], F32, tag="bd1")
                    bd2_sb = fw.tile([128, 8], F32, tag="bd2")
                    # per-chunk weight DMAs so the first matmuls start early
                    for kc in range(16):
                        nc.scalar.dma_start(
                            wd1_sb[:, kc, :],
                            wd1T.ap()[kc, :, h * M:(h + 1) * M],
                        )
                        if kc == 0:
                            load_dc(0)
                    for kc in range(8):
                        nc.scalar.dma_start(wd2_sb[:, kc, :],
                                            wd2T.ap()[h * 8 + kc])
                    nc.scalar.dma_start(bd1_sb, bd1.ap()[:, h * 8:(h + 1) * 8])
                    if h == 1:
                        nc.scalar.dma_start(bd2_sb, bd2.ap())

                    for rt in range(NTF):
                        rs = slice(rt * RF, (rt + 1) * RF)
                        if rt + 1 < NTF:
                            load_dc(rt + 1)
                        dc, wf = dcs.pop(rt)
                        pf = fd.tile([128, 24, RF], mm_dtype, tag="ptf",
                                     name="ptf", bufs=1)
                        nc.sync.dma_start(
                            pf, PT.rearrange("i c p r -> (i c) p r")[:, :, rs]
                            .rearrange("c p r -> p c r")
                        )
                        # ctx = sum_i ws_i * P_i, written into dc[:, 0:8]
                        shp = (128, 8, RF)
                        t2 = fo.tile([128, 8, RF], F32, tag="t2", name="t2")
                        t3 = fo.tile([128, 8, RF], F32, tag="t3", name="t3")
                        nc.vector.tensor_tensor(
                            dc[:, 0:8, :],
                            wf[:, 0, None, :].to_broadcast(shp),
                            pf[:, 0:8, :], AluOp.mult)
                        nc.vector.tensor_tensor(
                            t2, wf[:, 1, None, :].to_broadcast(shp),
                            pf[:, 8:16, :], AluOp.mult)
                        nc.gpsimd.tensor_tensor(
                            t3, wf[:, 2, None, :].to_broadcast(shp),
                            pf[:, 16:24, :], AluOp.mult)
                        nc.vector.tensor_tensor(
                            dc[:, 0:8, :], dc[:, 0:8, :], t2, AluOp.add)
                        nc.vector.tensor_tensor(
                            dc[:, 0:8, :], dc[:, 0:8, :], t3, AluOp.add)
                        o1 = fo.tile([128, 8, RF], mm_dtype)
                        for fc in range(8):
                            ps = psF.tile([128, RF], F32, tag="f1")
                            for kc in range(16):
                                nc.tensor.matmul(
                                    ps,
                                    wd1_sb[:, kc, fc * 128:(fc + 1) * 128],
                                    dc[:, kc, :],
                                    start=(kc == 0),
                                    stop=(kc == 15),
                                )
                            nc.scalar.activation(
                                o1[:, fc, :], ps, Act.Relu,
                                bias=bd1_sb[:, fc:fc + 1],
                            )
                        for oc in range(8):
                            ow = 128 if oc < 7 else OUT - 7 * 128
                            ps = psG.tile([128, RF], F32, tag="f2")
                            for kc in range(8):
                                nc.tensor.matmul(
                                    ps[:ow],
                                    wd2_sb[:, kc, oc * 128:oc * 128 + ow],
                                    o1[:, kc, :],
                                    start=(kc == 0),
                                    stop=(kc == 7),
                                )
                            ev = fe.tile([128, RF], F32)
                            if h == 1:
                                nc.vector.tensor_scalar_add(
                                    ev[:ow], ps[:ow], bd2_sb[:ow, oc:oc + 1]
                                )
                            else:
                                nc.vector.tensor_copy(ev[:ow], ps[:ow])
                            nc.sync.dma_start(
                                outH[h].ap()[oc * 128:oc * 128 + ow, rs],
                                ev[:ow],
                            )

    nc.finalize()
    return nc


def _prep_inputs(tube, w1_W, w1_b, w2_W, w2_b, w3_W, w3_b, wh_W, wh_b,
                 wd1_W, wd1_b, wd2_W, wd2_b):
    """Host-side reshape/transpose into the kernel's DRAM layouts."""
    f32 = np.float32

    def wT(w, kc):  # [F, K] -> [K, F] -> [kc, 128, F]
        w = np.asarray(w, f32)
        return np.ascontiguousarray(w.T).reshape(kc, 128, w.shape[0])

    def bmat(b, cc):  # [F] -> [128, cc]
        b = np.asarray(b, f32)
        if b.shape[0] < cc * 128:
            b = np.pad(b, (0, cc * 128 - b.shape[0]))
        return np.ascontiguousarray(b.reshape(cc, 128).T)

    shared = {
        "w1T": wT(w1_W, 4), "w2T": wT(w2_W, 4), "w3T": wT(w3_W, 4),
        "whT": wT(wh_W, 24), "wd1T": wT(wd1_W, 16), "wd2T": wT(wd2_W, 16),
        "b1": bmat(w1_b, 8), "b2": bmat(w2_b, 8), "b3": bmat(w3_b, 8),
        "bh": bmat(wh_b, 8), "bd1": bmat(wd1_b, 16), "bd2": bmat(wd2_b, 8),
    }
    tubeT = np.ascontiguousarray(np.asarray(tube, f32).T)  # [1536, B]
    in_maps = []
    for c in range(N_CORES):
        xTc = np.ascontiguousarray(
            tubeT[:, c * ROWS:(c + 1) * ROWS]
        ).reshape(12, 128, ROWS)
        in_maps.append({"xT": xTc, **shared})
    return in_maps


_NC_CACHE = {}


def run(inputs, mm_dtype=F32R, trace=False):
    key = (mm_dtype, )
    if key not in _NC_CACHE:
        _NC_CACHE[key] = build_nc(mm_dtype)
    nc = _NC_CACHE[key]
    in_maps = _prep_inputs(**inputs)
    res = run_bass_kernel_spmd(nc, in_maps, list(range(N_CORES)), trace=trace)
    out = np.empty((B, OUT), np.float32)
    for c in range(N_CORES):
        r = res.results[c]
        out[c * ROWS:(c + 1) * ROWS] = (r["out0"] + r["out1"]).T
    return out, res


def kernel(**inputs) -> np.ndarray:
    out, _ = run(inputs)
    return out

